# revision 1
# baseline (speedup 1.0000x reference)
"""Trainium2 Bass kernel for nn_DiffDelRNN (GRU + time-varying fractional delay).

v2 design (8 NeuronCores, data-parallel over batch N=32 -> 4 seqs/core):

GRU phase: T=65536 is split into F=2048 chunks of L=32 steps, run as 2
"pairs"; each pair partition-merges 2 batches of 512 chunks so every gate op
covers 128 (or 64) partitions at 512 columns. W=16 warmup steps per chunk
(validated: rel err ~2e-5). Per pair per step: one fp32r matmul computes all
sigmoid pre-activations [z_A z_B r_A r_B] (K=73: 64 h rows + 8 x rows + ones),
a second computes [ghn_A ghn_B gin_A gin_B]; ACT does sigmoid/tanh; DVE/Pool
do the 5-op gate algebra. Pred (w_out.h) is a third tiny matmul into a
2-bank psum tile, copied out every 2 steps and DMAed to a u-major DRAM
staging layout (contiguous 2KB runs, no transpose needed on device).

Delay phase: y[t] = (1-frac)*xpad[j] + frac*xpad[j+1], j = floor(1e4-dt)+t.
The staging tensor stg[g] holds [buffer | pred] in u-major order
(addr = u*2361 + (t+10016)//32); windows win[p=(g,a), 12064] are loaded with
an affine AP and gpsimd ap_gather (full-span window, d=1, 2048 idx/call,
host-permuted indices) gathers both taps; 32 calls x (8 valid rows each) are
re-assembled by partition-strided SBUF DMAs, combined by 3 DVE ops.

Self-contained: hardcodes all shapes; host-side prep is numpy only.
"""

import os

import ml_dtypes
import numpy as np

BF16 = ml_dtypes.bfloat16

N, C, T, H = 32, 1, 65536, 8
MAXD = 10000
NCORES = 8
GPC = 4               # sequences per core
NPAIR = 2             # chunk-pair pipelines per core
FB = 512              # chunk columns per batch (= matmul free dim)
FTOT = 2048           # total chunks per core (= NPAIR * 2 * FB)
L = T // FTOT         # 32 timesteps per chunk
W = 16                # warmup steps (validated ~2e-5 rel err)
S = L + W             # pipeline steps per pair
TS = 2048             # delay-phase timesteps per partition row
BUFC = 313            # buffer chunks of 32 (313*32 = 10016 >= MAXD+pad)
ROWF = BUFC + 64      # 377 window f-entries per u
WINC = 32 * ROWF      # 12064 window cols per partition
STGW = 32 * (BUFC + FTOT)   # 75552 per-seq staging width

MM_DT = os.environ.get("KBASS_MM_DT", "float32r")
NIDX = 2048           # ap_gather indices per call


def _build_gru_host(x, w_ih, w_hh, b_ih, b_hh, w_out):
    """lhsT matrices + per-core xr staging. x: (N, T) f32."""
    f32 = np.float32
    lhsTs = np.zeros((73, 128), f32)   # -> [z_A z_B r_A r_B]
    lhsTn = np.zeros((73, 128), f32)   # -> [ghn_A ghn_B gin_A gin_B]
    lhsTp = np.zeros((64, 8), f32)     # -> pred [A(seq0..3) B(seq0..3)]
    for b in range(2):                 # batch A/B within a pair
        for g in range(GPC):
            for i in range(H):
                m = b * 32 + g * 8 + i
                for j in range(H):
                    k = b * 32 + g * 8 + j
                    lhsTs[k, m] = w_hh[H + i, j]          # z
                    lhsTs[k, 64 + m] = w_hh[i, j]          # r
                    lhsTn[k, m] = w_hh[2 * H + i, j]       # ghn
                kx = 64 + b * 4 + g
                lhsTs[kx, m] = w_ih[H + i, 0]
                lhsTs[kx, 64 + m] = w_ih[i, 0]
                lhsTn[kx, 64 + m] = w_ih[2 * H + i, 0]     # gin
                lhsTs[72, m] = b_ih[H + i] + b_hh[H + i]
                lhsTs[72, 64 + m] = b_ih[i] + b_hh[i]
                lhsTn[72, m] = b_hh[2 * H + i]
                lhsTn[72, 64 + m] = b_ih[2 * H + i]
                lhsTp[b * 32 + g * 8 + i, b * 4 + g] = w_out[0, i]

    # xr[pair, row, s*FB + col]; rows 0:4 x_A(seq), 4:8 x_B(seq), 8 ones
    xr_cores = []
    for c in range(NCORES):
        xs = x[c * GPC:(c + 1) * GPC]                  # (4, T)
        xr = np.zeros((NPAIR, 9, S * FB), f32)
        xr[:, 8, :] = 1.0
        col = np.arange(FB)
        for pair in range(NPAIR):
            for b in range(2):
                f = (pair * 2 + b) * FB + col          # (FB,)
                for s in range(S):
                    t = f * L + s - W
                    v = np.where(t >= 0, xs[:, np.clip(t, 0, T - 1)], 0.0)
                    xr[pair, b * 4:(b + 1) * 4, s * FB:(s + 1) * FB] = v
        xr_cores.append(np.ascontiguousarray(
            xr.reshape(NPAIR * 9, S * FB).astype(BF16)))
    return lhsTs.astype(BF16), lhsTn.astype(BF16), lhsTp.astype(BF16), \
        xr_cores


def _build_delay_host(del_traj):
    """v1-style t-major gather plan: 3 sentinel sub-windows, uint16, pair-gather."""
    f32 = np.float32
    WCH = 4092
    wridx_c, frac_c = [], []
    for c in range(NCORES):
        dts = del_traj[c * GPC:(c + 1) * GPC]
        d = dts.reshape(GPC, 32, TS).reshape(128, TS)
        p = (np.float32(MAXD) - d).astype(f32)
        k0 = np.floor(p)
        frac_c.append((p - k0).astype(f32))
        jl = (k0.astype(np.int64) + np.arange(TS)[None, :])
        wr = np.zeros((128, 3 * TS), np.uint16)
        cw = TS // 16
        i = np.arange(TS)
        for w in range(3):
            b = w * WCH
            sw = np.where((jl >= b) & (jl < b + WCH), jl - b, 4094)
            sw = sw.astype(np.uint16)
            for k in range(16):
                for cc in range(8):
                    wr[16 * cc + (i % 16), w * TS + k * cw + i // 16] = \
                        sw[16 * cc + k, i]
        wridx_c.append(wr)
    return wridx_c, frac_c


def _build_program():
    import concourse.bacc as bacc
    import concourse.mybir as mybir
    import concourse.tile as tile
    from concourse.alu_op_type import AluOpType
    from concourse.ap import AP as _AP

    f32 = mybir.dt.float32
    bf16 = mybir.dt.bfloat16
    i16 = mybir.dt.int16
    ACT = mybir.ActivationFunctionType

    nc = bacc.Bacc("TRN2", target_bir_lowering=False, debug=False)

    # ---- I/O -------------------------------------------------------------
    xr_t = nc.dram_tensor("xr", [NPAIR * 9, S * FB], bf16,
                          kind="ExternalInput")
    lhsTs_t = nc.dram_tensor("lhsTs", [73, 128], bf16, kind="ExternalInput")
    lhsTn_t = nc.dram_tensor("lhsTn", [73, 128], bf16, kind="ExternalInput")
    lhsTp_t = nc.dram_tensor("lhsTp", [64, 8], bf16, kind="ExternalInput")
    buf_t = nc.dram_tensor("buf", [GPC, MAXD], f32, kind="ExternalInput")
    wridx_t = nc.dram_tensor("wridx", [128, 3 * TS], mybir.dt.uint16,
                             kind="ExternalInput")
    frac_t = nc.dram_tensor("frac", [128, TS], f32, kind="ExternalInput")
    pred_t = nc.dram_tensor("pred", [GPC, T], f32, kind="ExternalOutput")
    y_t = nc.dram_tensor("y", [GPC, T], f32, kind="ExternalOutput")
    XPAD = MAXD + T + 8

    with tile.TileContext(nc) as tc:
        import contextlib
        est = contextlib.ExitStack()
        gru_est = contextlib.ExitStack()
        with est:
            wpool = est.enter_context(tc.tile_pool(name="wpool", bufs=1))
            idxp = est.enter_context(tc.tile_pool(name="idxp", bufs=1))

            lts = wpool.tile([73, 128], bf16)
            ltn = wpool.tile([73, 128], bf16)
            ltp = wpool.tile([64, 8], bf16)
            nc.sync.dma_start(lts[:], lhsTs_t[:])
            nc.sync.dma_start(ltn[:], lhsTn_t[:])
            nc.sync.dma_start(ltp[:], lhsTp_t[:])

            wridx = idxp.tile([128, 3 * TS], mybir.dt.uint16, tag="wridx")
            frac = idxp.tile([128, TS], f32, tag="frac")
            nc.sync.dma_start(wridx[:], wridx_t[:])
            nc.sync.dma_start(frac[:], frac_t[:])

            dramp = est.enter_context(
                tc.tile_pool(name="dramp", bufs=1, space="DRAM"))
            xpad = dramp.tile([GPC, XPAD], f32)
            nc.sync.dma_start(xpad[:, 0:MAXD], buf_t[:])

            # ---- GRU phase ---------------------------------------------
            hxp = [gru_est.enter_context(
                tc.tile_pool(name=f"hx{p}", bufs=2)) for p in range(NPAIR)]
            psS = [gru_est.enter_context(
                tc.tile_pool(name=f"psS{p}", bufs=1, space="PSUM"))
                for p in range(NPAIR)]
            psN = [gru_est.enter_context(
                tc.tile_pool(name=f"psN{p}", bufs=1, space="PSUM"))
                for p in range(NPAIR)]
            psP = [gru_est.enter_context(
                tc.tile_pool(name=f"psP{p}", bufs=1, space="PSUM"))
                for p in range(NPAIR)]
            rzp = gru_est.enter_context(tc.tile_pool(name="rzp", bufs=2))
            up = gru_est.enter_context(tc.tile_pool(name="up", bufs=2))
            tp = gru_est.enter_context(tc.tile_pool(name="tp", bufs=2))
            np_ = gru_est.enter_context(tc.tile_pool(name="np", bufs=2))
            zp = gru_est.enter_context(tc.tile_pool(name="zp", bufs=2))
            qp = gru_est.enter_context(tc.tile_pool(name="qp", bufs=2))
            stp = gru_est.enter_context(tc.tile_pool(name="stp", bufs=1))

            pstage = []
            for p in range(NPAIR):
                pst = stp.tile([8, FB * L], f32, tag=f"pst{p}",
                               name=f"pst{p}")
                pstage.append(pst)

            hx = []
            for p in range(NPAIR):
                t0 = hxp[p].tile([73, FB], bf16, tag=f"hx{p}")
                nc.vector.memset(t0[0:64, :], 0.0)
                nc.sync.dma_start(t0[64:73, :],
                                  xr_t[p * 9:(p + 1) * 9, 0:FB])
                hx.append(t0)

            ppred = [None] * NPAIR
            for s in range(S):
                for p in range(NPAIR):
                    cur = hx[p]
                    nxt = hxp[p].tile([73, FB], bf16, tag=f"hx{p}")
                    if s + 1 < S:
                        nc.sync.dma_start(
                            nxt[64:73, :],
                            xr_t[p * 9:(p + 1) * 9,
                                 (s + 1) * FB:(s + 2) * FB])
                    ps = psS[p].tile([128, FB], f32, tag=f"psS{p}")
                    nc.tensor.matmul(ps[:], lts[:], cur[:],
                                     start=True, stop=True)
                    pn = psN[p].tile([128, FB], f32, tag=f"psN{p}")
                    nc.tensor.matmul(pn[:], ltn[:], cur[:],
                                     start=True, stop=True)
                    rz = rzp.tile([128, FB], bf16, tag=f"rz{p}")
                    nc.scalar.activation(rz[:], ps[:], ACT.Sigmoid)
                    u = up.tile([64, FB], bf16, tag=f"u{p}")
                    nc.vector.tensor_tensor(out=u[:], in0=rz[64:128, :],
                                            in1=pn[0:64, :],
                                            op=AluOpType.mult)
                    t2 = tp.tile([64, FB], bf16, tag=f"t2{p}")
                    nc.vector.tensor_tensor(out=t2[:], in0=u[:],
                                            in1=pn[64:128, :],
                                            op=AluOpType.add)
                    nn = np_.tile([64, FB], bf16, tag=f"nn{p}")
                    nc.scalar.activation(nn[:], t2[:], ACT.Tanh)
                    zh = zp.tile([64, FB], bf16, tag=f"zh{p}")
                    nc.gpsimd.tensor_tensor(out=zh[:], in0=rz[0:64, :],
                                            in1=cur[0:64, :],
                                            op=AluOpType.mult)
                    q = qp.tile([64, FB], bf16, tag=f"q{p}")
                    eng = nc.vector
                    eng.scalar_tensor_tensor(
                        out=q[:], in0=rz[0:64, :], scalar=1.0, in1=nn[:],
                        op0=AluOpType.subtract, op1=AluOpType.mult)
                    # q = (z - 1) * n, so h' = z*h + (1-z)*n = zh - q
                    nc.vector.tensor_tensor(out=nxt[0:64, :], in0=zh[:],
                                            in1=q[:], op=AluOpType.subtract)
                    if s == W - 1 and p == 0:
                        nc.vector.memset(nxt[0:32, 0:1], 0.0)
                    if s >= W:
                        uu = s - W
                        if uu % 2 == 0:
                            ppred[p] = psP[p].tile(
                                [8, 1024], f32, tag=f"psP{p}",
                                name=f"psP{p}")
                        pp = ppred[p]
                        nc.tensor.matmul(
                            pp[:, (uu % 2) * FB:(uu % 2 + 1) * FB],
                            ltp[:], nxt[0:64, :],
                            start=True, stop=True)
                        if uu % 2 == 1:
                            dstv = pstage[p][:, :].rearrange(
                                "r (f pb) -> r pb f", pb=L)[
                                :, uu - 1:uu + 1, :]
                            if p == 0:
                                nc.scalar.activation(dstv, pp[:], ACT.Copy)
                            else:
                                nc.vector.tensor_copy(dstv, pp[:])
                    hx[p] = nxt

            # pstage[p][(b g), f*L + uu] -> pred_t[g, ((p*2+b)*FB+f)*L+uu]
            for p in range(NPAIR):
                for b in range(2):
                    s3 = pstage[p][b * 4:(b + 1) * 4, :]
                    for (tt, off) in ((pred_t[:], 0), (xpad[:], MAXD)):
                        dst = _AP(tt.tensor,
                                  tt.offset + off + (p * 2 + b) * FB * L,
                                  [[XPAD if off else T, GPC],
                                   [L, FB], [1, L]])
                        nc.sync.dma_start(dst, s3)

            gru_est.close()
            tc.strict_bb_all_engine_barrier()

            # ---- delay phase -------------------------------------------
            WCH = 4092
            dpool = est.enter_context(tc.tile_pool(name="dpool", bufs=1))
            spool = est.enter_context(tc.tile_pool(name="spool", bufs=3))
            apool = est.enter_context(tc.tile_pool(name="apool", bufs=2))
            ypool = est.enter_context(tc.tile_pool(name="ypool", bufs=1))

            wins = []
            windma = []
            for w in range(3):
                wlen = min(WCH + 2, TS + MAXD + 2 - w * WCH)
                winw = dpool.tile([128, 4096], f32, tag=f"win{w}",
                                  name=f"win{w}")
                xap = xpad[:]
                wsrc = _AP(xap.tensor, xap.offset + w * WCH,
                           [[XPAD, GPC], [TS, 32], [1, wlen]])
                d1 = nc.sync.dma_start(winw[:, 0:wlen], wsrc)
                zw = nc.vector.memset(winw[:, 4094:4096], 0.0)
                wins.append(winw)
                windma.append((d1, zw))

            from concourse.tile import add_dep_helper as _adh
            pairs = ypool.tile([128, 2 * TS], f32, tag="pairs")
            CW = TS // 16
            NIB = 512
            cb = NIB // 16
            for k in range(16):
                for b in range(TS // NIB):
                    scrs = []
                    for w in range(3):
                        scr = spool.tile([128, NIB, 2], f32, tag=f"sc{w}",
                                         name=f"sc{w}")
                        c0 = w * TS + CW * k + cb * b
                        gi = nc.gpsimd.add_instruction(
                            mybir.InstIndirectCopy(
                                name=f"I-{nc.next_id()}",
                                ins=[nc.gpsimd.lower_ap(wins[w][:]),
                                     nc.gpsimd.lower_ap(
                                         wridx[:, c0:c0 + cb])],
                                outs=[nc.gpsimd.lower_ap(scr[:])],
                                num_valid_indices=NIB,
                            ))
                        _adh(gi.ins, windma[w][0].ins, sync=True, reason="g")
                        _adh(gi.ins, windma[w][1].ins, sync=True, reason="g")
                        scrs.append((scr, gi))
                    s01 = apool.tile([128, 2 * NIB], f32, tag="s01")
                    a1 = nc.vector.tensor_tensor(
                        out=s01[:],
                        in0=scrs[0][0][:].rearrange("p u e -> p (u e)"),
                        in1=scrs[1][0][:].rearrange("p u e -> p (u e)"),
                        op=AluOpType.add)
                    ssum = apool.tile([128, 2 * NIB], f32, tag="ssum")
                    a2 = nc.vector.tensor_tensor(
                        out=ssum[:], in0=s01[:],
                        in1=scrs[2][0][:].rearrange("p u e -> p (u e)"),
                        op=AluOpType.add)
                    for _, gi in scrs:
                        _adh(a1.ins, gi.ins, sync=True, reason="s")
                        _adh(a2.ins, gi.ins, sync=True, reason="s")
                    ed = nc.sync.dma_start(
                        pairs[:][k:k + 113:16, b * 2 * NIB:(b + 1) * 2 * NIB],
                        ssum[:][k:k + 113:16, :])
                    _adh(ed.ins, a2.ins, sync=True, reason="e")

            y0 = pairs[:].rearrange("p (u e) -> p u e", e=2)[:, :, 0:1]\
                .rearrange("p u e -> p (u e)")
            y1 = pairs[:].rearrange("p (u e) -> p u e", e=2)[:, :, 1:2]\
                .rearrange("p u e -> p (u e)")
            q0 = ypool.tile([128, TS], f32, tag="q0")
            nc.vector.scalar_tensor_tensor(
                out=q0[:], in0=frac[:], scalar=1.0, in1=y0,
                op0=AluOpType.subtract, op1=AluOpType.mult)
            q1 = ypool.tile([128, TS], f32, tag="q1")
            nc.vector.tensor_tensor(out=q1[:], in0=frac[:], in1=y1,
                                    op=AluOpType.mult)
            yt = ypool.tile([128, TS], f32, tag="yt")
            nc.vector.tensor_tensor(out=yt[:], in0=q1[:], in1=q0[:],
                                    op=AluOpType.subtract)
            nc.sync.dma_start(
                y_t[:].rearrange("g (a u) -> (g a) u", a=32), yt[:])

    nc.compile()
    return nc


_NC_CACHE = {}


def kernel(x, del_traj, buffer, w_ih, w_hh, b_ih, b_hh, w_out):
    from concourse.bass_utils import run_bass_kernel_spmd

    x = np.asarray(x, np.float32).reshape(N, T)
    del_traj = np.asarray(del_traj, np.float32).reshape(N, T)
    buffer = np.asarray(buffer, np.float32).reshape(N, MAXD)
    w_ih = np.asarray(w_ih, np.float32)
    w_hh = np.asarray(w_hh, np.float32)
    b_ih = np.asarray(b_ih, np.float32)
    b_hh = np.asarray(b_hh, np.float32)
    w_out = np.asarray(w_out, np.float32)

    lhsTs, lhsTn, lhsTp, xr_cores = _build_gru_host(
        x, w_ih, w_hh, b_ih, b_hh, w_out)
    wridx_c, frac_c = _build_delay_host(del_traj)

    if "nc" not in _NC_CACHE:
        _NC_CACHE["nc"] = _build_program()
    nc = _NC_CACHE["nc"]

    in_maps = []
    for c in range(NCORES):
        sl = slice(c * GPC, (c + 1) * GPC)
        in_maps.append({
            "xr": xr_cores[c],
            "lhsTs": lhsTs, "lhsTn": lhsTn, "lhsTp": lhsTp,
            "buf": np.ascontiguousarray(buffer[sl]),
            "wridx": wridx_c[c], "frac": frac_c[c],
        })

    trace = bool(int(os.environ.get("KBASS_TRACE", "0")))
    tmpdir = os.environ.get("KBASS_TMPDIR") or None
    try:
        res = run_bass_kernel_spmd(nc, in_maps, list(range(NCORES)),
                                   trace=trace, tmpdir=tmpdir)
    except ModuleNotFoundError:
        res = run_bass_kernel_spmd(nc, in_maps, list(range(NCORES)))
    if res.exec_time_ns is not None:
        print(f"HW exec time: {res.exec_time_ns} ns")
    if res.instructions_and_trace is not None:
        print(f"trace path: {res.instructions_and_trace[1]}")

    y = np.zeros((N, 1, T), np.float32)
    pred = np.zeros((N, 1, T), np.float32)
    for c in range(NCORES):
        sl = slice(c * GPC, (c + 1) * GPC)
        y[sl, 0, :] = res.results[c]["y"]
        pred[sl, 0, :] = res.results[c]["pred"]
    return (y, pred)



# revision 8
# speedup vs baseline: 1.7240x; 1.7240x over previous
"""Trainium2 Bass kernel for nn_DiffDelRNN (GRU + time-varying fractional delay).

v3 design (8 NeuronCores, data-parallel over batch N=32 -> 4 seqs/core):

GRU phase (unchanged from v2): T=65536 split into F=2048 chunks of L=32
steps, run as 2 "pairs"; each pair partition-merges 2 batches of 512 chunks so
every gate op covers 128 (or 64) partitions at 512 columns. W=16 warmup steps
per chunk. Per pair per step: one matmul computes all sigmoid pre-activations
[z_A z_B r_A r_B] (K=73), a second [ghn_A ghn_B gin_A gin_B]; ACT does
sigmoid/tanh; DVE/Pool do the gate algebra. Pred (w_out.h) is a third tiny
matmul copied out every 2 steps and DMAed to u-major DRAM staging.

Delay phase (v3 rewrite): the v2 design used 192 gpsimd InstIndirectCopy
calls + 64 strided SBUF->SBUF reassembly DMAs; on HW every INDIRECT_COPY
carries a ~15us launch overhead, so the delay phase alone took ~3.3ms.
v3 uses 64 rows x 4096 samples per core, ONE full-span window per row
(win[128,14112] f32, real rows at partitions 16cc+k, k<8), and only 8
gather calls with num_valid_indices=4096. Valid rows (one per gpsimd core
group) are moved into the final pairs layout by partition-strided
scalar/vector-engine copies (no DMA), then 3 DVE ops apply the fractional
weights.

Self-contained: hardcodes all shapes; host-side prep is numpy only.
"""

import os

import ml_dtypes
import numpy as np

BF16 = ml_dtypes.bfloat16

N, C, T, H = 32, 1, 65536, 8
MAXD = 10000
NCORES = 8
GPC = 4               # sequences per core
NPAIR = 2             # chunk-pair pipelines per core
FB = 512              # chunk columns per batch (= matmul free dim)
FTOT = 2048           # total chunks per core (= NPAIR * 2 * FB)
L = T // FTOT         # 32 timesteps per chunk
W = 16                # warmup steps (validated ~2e-5 rel err)
S = L + W             # pipeline steps per pair
TS = 2048             # final-layout timesteps per partition row

# delay phase v3 geometry
RLEN = 4096           # samples per gather row (64 rows per core)
WLEN = 14112          # window f32 per row (RLEN + MAXD + slack)
NIDX = 2 * RLEN       # ap_gather indices per call (both taps)
CIDX = NIDX // 16     # idx columns per call (= 512)
XPAD = MAXD + T + 24  # 75560


def _build_gru_host(x, w_ih, w_hh, b_ih, b_hh, w_out):
    """lhsT matrices + per-core xr staging. x: (N, T) f32."""
    f32 = np.float32
    lhsTs = np.zeros((73, 128), f32)   # -> [z_A z_B r_A r_B]
    lhsTn = np.zeros((73, 128), f32)   # -> [ghn_A ghn_B gin_A gin_B]
    lhsTp = np.zeros((64, 8), f32)     # -> pred [A(seq0..3) B(seq0..3)]
    for b in range(2):                 # batch A/B within a pair
        for g in range(GPC):
            for i in range(H):
                m = b * 32 + g * 8 + i
                for j in range(H):
                    k = b * 32 + g * 8 + j
                    lhsTs[k, m] = w_hh[H + i, j]          # z
                    lhsTs[k, 64 + m] = w_hh[i, j]          # r
                    lhsTn[k, m] = w_hh[2 * H + i, j]       # ghn
                kx = 64 + b * 4 + g
                lhsTs[kx, m] = w_ih[H + i, 0]
                lhsTs[kx, 64 + m] = w_ih[i, 0]
                lhsTn[kx, 64 + m] = w_ih[2 * H + i, 0]     # gin
                lhsTs[72, m] = b_ih[H + i] + b_hh[H + i]
                lhsTs[72, 64 + m] = b_ih[i] + b_hh[i]
                lhsTn[72, m] = b_hh[2 * H + i]
                lhsTn[72, 64 + m] = b_ih[2 * H + i]
                lhsTp[b * 32 + g * 8 + i, b * 4 + g] = w_out[0, i]

    # xr[pair, row, s*FB + col]; rows 0:4 x_A(seq), 4:8 x_B(seq), 8 ones
    xr_cores = []
    for c in range(NCORES):
        xs = x[c * GPC:(c + 1) * GPC]                  # (4, T)
        xr = np.zeros((NPAIR, 9, S * FB), f32)
        xr[:, 8, :] = 1.0
        col = np.arange(FB)
        for pair in range(NPAIR):
            for b in range(2):
                f = (pair * 2 + b) * FB + col          # (FB,)
                for s in range(S):
                    t = f * L + s - W
                    v = np.where(t >= 0, xs[:, np.clip(t, 0, T - 1)], 0.0)
                    xr[pair, b * 4:(b + 1) * 4, s * FB:(s + 1) * FB] = v
        xr_cores.append(np.ascontiguousarray(
            xr.reshape(NPAIR * 9, S * FB).astype(BF16)))
    return lhsTs.astype(BF16), lhsTn.astype(BF16), lhsTp.astype(BF16), \
        xr_cores


def _build_delay_host(del_traj):
    """v3 gather plan: 64 rows x 4096 samples, full-span window, 8 ap_gather
    calls of NIDX=8192 both-tap indices (d=1).

    wridx[16cc+p, k*CIDX + s] = jl2[8cc+k, 16s+p]  (i16, row-local index)
    where jl2[r, 2i+e] = floor(1e4-dt) + i + e.
    frac stays in the final (g,a)-row layout [128, 2048]."""
    f32 = np.float32
    wridx_c, frac_c = [], []
    for c in range(NCORES):
        dts = del_traj[c * GPC:(c + 1) * GPC]          # (4, T)
        # gather-row layout (64, RLEN)
        d64 = dts.reshape(GPC, 16, RLEN).reshape(64, RLEN)
        p64 = (np.float32(MAXD) - d64).astype(f32)
        k064 = np.floor(p64)
        jl = (k064.astype(np.int64) + np.arange(RLEN)[None, :])  # (64, RLEN)
        jl2 = np.repeat(jl, 2, axis=1)
        jl2[:, 1::2] += 1                              # (64, NIDX)
        wr = np.zeros((128, 8 * CIDX), np.int16)
        # for k, cc: wr[16cc+p, k*CIDX+s] = jl2[8cc+k, 16s+p]
        j4 = jl2.reshape(8, 8, CIDX, 16)       # [cc, k, s, p]
        wr.reshape(8, 16, 8, CIDX)[:, :, :, :] = \
            j4.transpose(0, 3, 1, 2).astype(np.int16)
        wridx_c.append(wr)
        # final-layout frac (128, 2048)
        dfin = dts.reshape(GPC, 32, TS).reshape(128, TS)
        pfin = (np.float32(MAXD) - dfin).astype(f32)
        frac_c.append((pfin - np.floor(pfin)).astype(f32))
    return wridx_c, frac_c


def _build_program():
    import concourse.bacc as bacc
    import concourse.mybir as mybir
    import concourse.tile as tile
    from concourse.alu_op_type import AluOpType
    from concourse.ap import AP as _AP

    f32 = mybir.dt.float32
    bf16 = mybir.dt.bfloat16
    ACT = mybir.ActivationFunctionType

    nc = bacc.Bacc("TRN2", target_bir_lowering=False, debug=False)

    # ---- I/O -------------------------------------------------------------
    xr_t = nc.dram_tensor("xr", [NPAIR * 9, S * FB], bf16,
                          kind="ExternalInput")
    lhsTs_t = nc.dram_tensor("lhsTs", [73, 128], bf16, kind="ExternalInput")
    lhsTn_t = nc.dram_tensor("lhsTn", [73, 128], bf16, kind="ExternalInput")
    lhsTp_t = nc.dram_tensor("lhsTp", [64, 8], bf16, kind="ExternalInput")
    buf_t = nc.dram_tensor("buf", [GPC, MAXD], f32, kind="ExternalInput")
    wridx_t = nc.dram_tensor("wridx", [128, 8 * CIDX], mybir.dt.int16,
                             kind="ExternalInput")
    frac_t = nc.dram_tensor("frac", [128, TS], f32, kind="ExternalInput")
    pred_t = nc.dram_tensor("pred", [GPC, T], f32, kind="ExternalOutput")
    y_t = nc.dram_tensor("y", [GPC, T], f32, kind="ExternalOutput")

    with tile.TileContext(nc) as tc:
        import contextlib
        est = contextlib.ExitStack()
        gru_est = contextlib.ExitStack()
        with est:
            wpool = est.enter_context(tc.tile_pool(name="wpool", bufs=1))
            idxp = est.enter_context(tc.tile_pool(name="idxp", bufs=1))

            lts = wpool.tile([73, 128], bf16)
            ltn = wpool.tile([73, 128], bf16)
            ltp = wpool.tile([64, 8], bf16)
            nc.sync.dma_start(lts[:], lhsTs_t[:])
            nc.sync.dma_start(ltn[:], lhsTn_t[:])
            nc.sync.dma_start(ltp[:], lhsTp_t[:])

            wridx = idxp.tile([128, 8 * CIDX], mybir.dt.int16, tag="wridx")
            frac = idxp.tile([128, TS], f32, tag="frac")
            widma = nc.sync.dma_start(wridx[:], wridx_t[:])
            nc.sync.dma_start(frac[:], frac_t[:])

            dramp = est.enter_context(
                tc.tile_pool(name="dramp", bufs=1, space="DRAM"))
            xpad = dramp.tile([GPC, XPAD], f32)
            nc.sync.dma_start(xpad[:, 0:MAXD], buf_t[:])

            # ---- GRU phase ---------------------------------------------
            hxp = [gru_est.enter_context(
                tc.tile_pool(name=f"hx{p}", bufs=2)) for p in range(NPAIR)]
            psS = [gru_est.enter_context(
                tc.tile_pool(name=f"psS{p}", bufs=1, space="PSUM"))
                for p in range(NPAIR)]
            psN = [gru_est.enter_context(
                tc.tile_pool(name=f"psN{p}", bufs=1, space="PSUM"))
                for p in range(NPAIR)]
            psP = [gru_est.enter_context(
                tc.tile_pool(name=f"psP{p}", bufs=1, space="PSUM"))
                for p in range(NPAIR)]
            rzp = gru_est.enter_context(tc.tile_pool(name="rzp", bufs=2))
            up = gru_est.enter_context(tc.tile_pool(name="up", bufs=2))
            tp = gru_est.enter_context(tc.tile_pool(name="tp", bufs=2))
            np_ = gru_est.enter_context(tc.tile_pool(name="np", bufs=2))
            zp = gru_est.enter_context(tc.tile_pool(name="zp", bufs=2))
            qp = gru_est.enter_context(tc.tile_pool(name="qp", bufs=2))
            stp = gru_est.enter_context(tc.tile_pool(name="stp", bufs=1))

            pstage = []
            for p in range(NPAIR):
                pst = stp.tile([8, FB * L], f32, tag=f"pst{p}",
                               name=f"pst{p}")
                pstage.append(pst)

            hx = []
            for p in range(NPAIR):
                t0 = hxp[p].tile([73, FB], bf16, tag=f"hx{p}")
                nc.vector.memset(t0[0:64, :], 0.0)
                nc.sync.dma_start(t0[64:73, :],
                                  xr_t[p * 9:(p + 1) * 9, 0:FB])
                hx.append(t0)

            ppred = [None] * NPAIR
            for s in range(S):
                for p in range(NPAIR):
                    cur = hx[p]
                    nxt = hxp[p].tile([73, FB], bf16, tag=f"hx{p}")
                    if s + 1 < S:
                        nc.sync.dma_start(
                            nxt[64:73, :],
                            xr_t[p * 9:(p + 1) * 9,
                                 (s + 1) * FB:(s + 2) * FB])
                    ps = psS[p].tile([128, FB], f32, tag=f"psS{p}")
                    nc.tensor.matmul(ps[:], lts[:], cur[:],
                                     start=True, stop=True)
                    pn = psN[p].tile([128, FB], f32, tag=f"psN{p}")
                    nc.tensor.matmul(pn[:], ltn[:], cur[:],
                                     start=True, stop=True)
                    rz = rzp.tile([128, FB], bf16, tag=f"rz{p}")
                    nc.scalar.activation(rz[:], ps[:], ACT.Sigmoid)
                    u = up.tile([64, FB], bf16, tag=f"u{p}")
                    nc.vector.tensor_tensor(out=u[:], in0=rz[64:128, :],
                                            in1=pn[0:64, :],
                                            op=AluOpType.mult)
                    t2 = tp.tile([64, FB], bf16, tag=f"t2{p}")
                    nc.vector.tensor_tensor(out=t2[:], in0=u[:],
                                            in1=pn[64:128, :],
                                            op=AluOpType.add)
                    nn = np_.tile([64, FB], bf16, tag=f"nn{p}")
                    nc.scalar.activation(nn[:], t2[:], ACT.Tanh)
                    zh = zp.tile([64, FB], bf16, tag=f"zh{p}")
                    nc.gpsimd.tensor_tensor(out=zh[:], in0=rz[0:64, :],
                                            in1=cur[0:64, :],
                                            op=AluOpType.mult)
                    q = qp.tile([64, FB], bf16, tag=f"q{p}")
                    eng = nc.vector
                    eng.scalar_tensor_tensor(
                        out=q[:], in0=rz[0:64, :], scalar=1.0, in1=nn[:],
                        op0=AluOpType.subtract, op1=AluOpType.mult)
                    # q = (z - 1) * n, so h' = z*h + (1-z)*n = zh - q
                    nc.vector.tensor_tensor(out=nxt[0:64, :], in0=zh[:],
                                            in1=q[:], op=AluOpType.subtract)
                    if s == W - 1 and p == 0:
                        nc.vector.memset(nxt[0:32, 0:1], 0.0)
                    if s >= W:
                        uu = s - W
                        if uu % 2 == 0:
                            ppred[p] = psP[p].tile(
                                [8, 1024], f32, tag=f"psP{p}",
                                name=f"psP{p}")
                        pp = ppred[p]
                        nc.tensor.matmul(
                            pp[:, (uu % 2) * FB:(uu % 2 + 1) * FB],
                            ltp[:], nxt[0:64, :],
                            start=True, stop=True)
                        if uu % 2 == 1:
                            dstv = pstage[p][:, :].rearrange(
                                "r (f pb) -> r pb f", pb=L)[
                                :, uu - 1:uu + 1, :]
                            if p == 0:
                                nc.scalar.activation(dstv, pp[:], ACT.Copy)
                            else:
                                nc.vector.tensor_copy(dstv, pp[:])
                    hx[p] = nxt

            # pstage[p][(b g), f*L + uu] -> pred_t[g, ((p*2+b)*FB+f)*L+uu]
            for p in range(NPAIR):
                for b in range(2):
                    s3 = pstage[p][b * 4:(b + 1) * 4, :]
                    for (tt, off) in ((pred_t[:], 0), (xpad[:], MAXD)):
                        dst = _AP(tt.tensor,
                                  tt.offset + off + (p * 2 + b) * FB * L,
                                  [[XPAD if off else T, GPC],
                                   [L, FB], [1, L]])
                        nc.sync.dma_start(dst, s3)

            gru_est.close()
            tc.strict_bb_all_engine_barrier()

            # ---- delay phase (v3) --------------------------------------
            dpool = est.enter_context(tc.tile_pool(name="dpool", bufs=1))
            spool = est.enter_context(tc.tile_pool(name="spool", bufs=2))
            ypool = est.enter_context(tc.tile_pool(name="ypool", bufs=1))

            # window: win[16cc+k, :] = xpad[g, b4*RLEN : +WLEN], r=8cc+k,
            # g=r//16, b4=r%16.  8 DMAs, one per k (8 rows stride 16).
            win = dpool.tile([128, WLEN], f32, tag="win", name="win")
            windma = []
            xap = xpad[:]
            for k in range(8):
                wsrc = _AP(xap.tensor, xap.offset + k * RLEN,
                           [[XPAD, GPC], [8 * RLEN, 2], [1, WLEN]])
                dk = nc.sync.dma_start(win[:][k:k + 113:16, 0:WLEN], wsrc)
                windma.append(dk)

            from concourse.tile import add_dep_helper as _adh
            pairs = ypool.tile([128, 2 * TS], f32, tag="pairs")
            cp_hist = []          # copy instrs per call, for WAR deps
            for k in range(8):
                scr = spool.tile([128, NIDX], f32, tag="scr", name="scr")
                gi = nc.gpsimd.ap_gather(
                    scr[:], win[:], wridx[:, k * CIDX:(k + 1) * CIDX],
                    channels=128, num_elems=WLEN, d=1, num_idxs=NIDX)
                _adh(gi.ins, windma[k].ins, sync=True, reason="g")
                _adh(gi.ins, widma.ins, sync=True, reason="g")
                if k >= 2:              # scr WAR: bufs=2
                    for cp in cp_hist[k - 2]:
                        _adh(gi.ins, cp.ins, sync=True, reason="w")
                cps = []
                for blk in range(2):
                    src = scr[:][k:k + 113:16,
                                 blk * 2 * TS:(blk + 1) * 2 * TS]
                    dst = pairs[:][2 * k + blk:2 * k + blk + 113:16,
                                   0:2 * TS]
                    cp = nc.sync.dma_start(dst, src)
                    _adh(cp.ins, gi.ins, sync=True, reason="c")
                    cps.append(cp)
                cp_hist.append(cps)

            y0 = pairs[:].rearrange("p (u e) -> p u e", e=2)[:, :, 0:1]\
                .rearrange("p u e -> p (u e)")
            y1 = pairs[:].rearrange("p (u e) -> p u e", e=2)[:, :, 1:2]\
                .rearrange("p u e -> p (u e)")
            q0 = ypool.tile([128, TS], f32, tag="q0")
            c1 = nc.vector.scalar_tensor_tensor(
                out=q0[:], in0=frac[:], scalar=1.0, in1=y0,
                op0=AluOpType.subtract, op1=AluOpType.mult)
            q1 = ypool.tile([128, TS], f32, tag="q1")
            c2 = nc.vector.tensor_tensor(out=q1[:], in0=frac[:], in1=y1,
                                         op=AluOpType.mult)
            for cps in cp_hist:
                for cp in cps:
                    _adh(c1.ins, cp.ins, sync=True, reason="f")
                    _adh(c2.ins, cp.ins, sync=True, reason="f")
            yt = ypool.tile([128, TS], f32, tag="yt")
            nc.vector.tensor_tensor(out=yt[:], in0=q1[:], in1=q0[:],
                                    op=AluOpType.subtract)
            nc.sync.dma_start(
                y_t[:].rearrange("g (a u) -> (g a) u", a=32), yt[:])

    nc.compile()
    return nc


_NC_CACHE = {}


def kernel(x, del_traj, buffer, w_ih, w_hh, b_ih, b_hh, w_out):
    from concourse.bass_utils import run_bass_kernel_spmd

    x = np.asarray(x, np.float32).reshape(N, T)
    del_traj = np.asarray(del_traj, np.float32).reshape(N, T)
    buffer = np.asarray(buffer, np.float32).reshape(N, MAXD)
    w_ih = np.asarray(w_ih, np.float32)
    w_hh = np.asarray(w_hh, np.float32)
    b_ih = np.asarray(b_ih, np.float32)
    b_hh = np.asarray(b_hh, np.float32)
    w_out = np.asarray(w_out, np.float32)

    lhsTs, lhsTn, lhsTp, xr_cores = _build_gru_host(
        x, w_ih, w_hh, b_ih, b_hh, w_out)
    wridx_c, frac_c = _build_delay_host(del_traj)

    if "nc" not in _NC_CACHE:
        _NC_CACHE["nc"] = _build_program()
    nc = _NC_CACHE["nc"]

    in_maps = []
    for c in range(NCORES):
        sl = slice(c * GPC, (c + 1) * GPC)
        in_maps.append({
            "xr": xr_cores[c],
            "lhsTs": lhsTs, "lhsTn": lhsTn, "lhsTp": lhsTp,
            "buf": np.ascontiguousarray(buffer[sl]),
            "wridx": wridx_c[c], "frac": frac_c[c],
        })

    trace = bool(int(os.environ.get("KBASS_TRACE", "0")))
    tmpdir = os.environ.get("KBASS_TMPDIR") or None
    try:
        res = run_bass_kernel_spmd(nc, in_maps, list(range(NCORES)),
                                   trace=trace, tmpdir=tmpdir)
    except ModuleNotFoundError:
        res = run_bass_kernel_spmd(nc, in_maps, list(range(NCORES)))
    if res.exec_time_ns is not None:
        print(f"HW exec time: {res.exec_time_ns} ns")
    if res.instructions_and_trace is not None:
        print(f"trace path: {res.instructions_and_trace[1]}")

    y = np.zeros((N, 1, T), np.float32)
    pred = np.zeros((N, 1, T), np.float32)
    for c in range(NCORES):
        sl = slice(c * GPC, (c + 1) * GPC)
        y[sl, 0, :] = res.results[c]["y"]
        pred[sl, 0, :] = res.results[c]["pred"]
    return (y, pred)
